# revision 6
# baseline (speedup 1.0000x reference)
"""Grouped GEMM (MoE expert-parallel) on 8 TRN2 NeuronCores.

Strategy: expert-parallel — core e computes Y_e = X_e @ W_e^T for its expert's
contiguous token group.  Per core: [2048, 1024] @ [1024, 2048] -> [2048, 2048].

All HBM traffic is bf16 (the 2e-2 rel-err budget dwarfs bf16's ~4e-3):
  xt 4 MB + wt 4 MB in, y 8 MB out = 16 MB/core vs 32 MB for fp32.
Matmuls accumulate in fp32 PSUM; the PSUM->SBUF copy casts to bf16.
Host side packs operands so every DMA line is contiguous per partition,
runs the SPMD Bass kernel, and scatters per-expert results back.
"""

import numpy as np

import concourse.mybir as mybir
import concourse.tile as tile
from concourse import bacc

NUM_CORES = 8
IN_F = 1024            # K (contraction)
OUT_F = 2048           # N (out features per expert)
CAP = 2048             # token capacity per core (= expected group size)
P = 128
KT = IN_F // P         # 8 k-subtiles
MT = CAP // P          # 16 m-tiles of tokens
NFREE = 512            # moving-operand free dim (one fp32 PSUM bank)
NT = OUT_F // NFREE    # 4 n-tiles

MM_DT = mybir.dt.bfloat16
Y_DT = mybir.dt.bfloat16


def _np_bf16():
    import ml_dtypes
    return np.dtype(ml_dtypes.bfloat16)


def _build(repeat: int = 1):
    """Build the per-core Bass program: y[CAP, OUT_F] = X_e @ W_e^T.

    xt: [P, MT, KT, P]        xt[p, mt, o, j] = X[mt*128 + j, o*128 + p]
    wt: [P, NT, KT, NFREE]    wt[p, n, o, j]  = W[n*512 + j, o*128 + p]
    (both bf16, per-partition contiguous strips for efficient DMA)
    """
    nc = bacc.Bacc(None, target_bir_lowering=False, debug=False)
    xt = nc.dram_tensor("xt", [P, MT, KT, P], MM_DT, kind="ExternalInput")
    wt = nc.dram_tensor("wt", [P, NT, KT, NFREE], MM_DT, kind="ExternalInput")
    y = nc.dram_tensor("y", [CAP, OUT_F], Y_DT, kind="ExternalOutput")

    yr = y.rearrange("(mt p) n -> p mt n", p=P)   # [128, MT, OUT_F]

    with tile.TileContext(nc) as tc:
        with (
            tc.tile_pool(name="xt_pool", bufs=2) as xt_pool,
            tc.tile_pool(name="wt_pool", bufs=2) as wt_pool,
            tc.tile_pool(name="out_pool", bufs=4) as out_pool,
            tc.tile_pool(name="psum", bufs=8, space="PSUM") as psum_pool,
        ):
            for _ in range(repeat):
                # whole wt + xt resident in SBUF, one 4 MB DMA each; bufs=2
                # lets the next iteration's loads overlap this one's matmuls.
                wt_t = wt_pool.tile([P, NT, KT, NFREE], MM_DT, tag="wt")
                nc.sync.dma_start(wt_t[:], wt[:])
                xt_t = xt_pool.tile([P, MT, KT, P], MM_DT, tag="xt")
                nc.sync.dma_start(xt_t[:], xt[:])
                for m in range(MT):
                    y_sb = out_pool.tile([P, OUT_F], Y_DT, tag="y")
                    psums = [
                        psum_pool.tile(
                            [P, NFREE], mybir.dt.float32,
                            name=f"psum_{m}_{n}", tag="psum",
                        )
                        for n in range(NT)
                    ]
                    # n-outer issue order: each PSUM group's 8 accumulating
                    # matmuls go back-to-back on one bank.
                    for n in range(NT):
                        for o in range(KT):
                            nc.tensor.matmul(
                                psums[n],
                                lhsT=xt_t[:, m, o, :],
                                rhs=wt_t[:, n, o, :],
                                start=(o == 0),
                                stop=(o == KT - 1),
                            )
                    # drain PSUM; let the scheduler balance DVE/Act engines
                    for n in range(NT):
                        nc.any.tensor_copy(
                            y_sb[:, n * NFREE:(n + 1) * NFREE], psums[n][:]
                        )
                    nc.sync.dma_start(yr[:, m, :], y_sb[:])
    nc.compile()
    return nc


_NC_CACHE: dict = {}


def _get_nc(repeat: int = 1):
    if repeat not in _NC_CACHE:
        _NC_CACHE[repeat] = _build(repeat)
    return _NC_CACHE[repeat]


_RUNNER_CACHE: dict = {}


def _get_runner():
    """Jit the 8-core SPMD executable once; reuse across kernel() calls."""
    if "run" in _RUNNER_CACHE:
        return _RUNNER_CACHE["run"]

    import jax
    from jax.sharding import Mesh, PartitionSpec
    from jax.experimental.shard_map import shard_map
    from concourse import bass2jax
    from concourse.bass2jax import _bass_exec_p, install_neuronx_cc_hook

    nc = _get_nc(1)
    install_neuronx_cc_hook()
    assert nc.dbg_addr is None, "rebuild with debug=False"
    partition_name = (
        nc.partition_id_tensor.name if nc.partition_id_tensor else None
    )

    in_names, out_names, out_avals = [], [], []
    for alloc in nc.m.functions[0].allocations:
        if not isinstance(alloc, mybir.MemoryLocationSet):
            continue
        name = alloc.memorylocations[0].name
        if alloc.kind == "ExternalInput":
            if name != partition_name:
                in_names.append(name)
        elif alloc.kind == "ExternalOutput":
            out_names.append(name)
            out_avals.append(
                jax.core.ShapedArray(
                    tuple(alloc.tensor_shape), mybir.dt.np(alloc.dtype)
                )
            )
    n_params = len(in_names)
    all_in_names = list(in_names) + list(out_names)
    if partition_name is not None:
        all_in_names.append(partition_name)
    donate = tuple(range(n_params, n_params + len(out_names)))

    def _body(*args):
        operands = list(args)
        if partition_name is not None:
            operands.append(bass2jax.partition_id_tensor())
        outs = _bass_exec_p.bind(
            *operands,
            out_avals=tuple(out_avals),
            in_names=tuple(all_in_names),
            out_names=tuple(out_names),
            lowering_input_output_aliases=(),
            sim_require_finite=True,
            sim_require_nnan=True,
            nc=nc,
        )
        return tuple(outs)

    devices = jax.devices()[:NUM_CORES]
    mesh = Mesh(np.asarray(devices), ("core",))
    spec = PartitionSpec("core")
    fn = jax.jit(
        shard_map(
            _body, mesh=mesh,
            in_specs=(spec,) * (n_params + len(out_names)),
            out_specs=(spec,) * len(out_names),
            check_rep=False,
        ),
        donate_argnums=donate, keep_unused=True,
    )

    def run(in_maps):
        concat_in = [
            np.concatenate([np.asarray(m[k]) for m in in_maps], axis=0)
            for k in in_names
        ]
        zeros = [
            np.zeros((NUM_CORES * a.shape[0], *a.shape[1:]), a.dtype)
            for a in out_avals
        ]
        outs = fn(*concat_in, *zeros)
        arr = np.asarray(outs[0]).reshape(NUM_CORES, *out_avals[0].shape)
        return [{out_names[0]: arr[c]} for c in range(NUM_CORES)]

    _RUNNER_CACHE["run"] = run
    return run


def _chunk_in_map(x, w, off: int, size: int, expert: int):
    """Build the per-core input map for one (expert, token-chunk).

    x, w are fp32 arrays; packing + bf16 cast happens here.
    """
    bf16 = _np_bf16()
    xe = np.zeros((CAP, IN_F), np.float32)
    if size > 0:
        xe[:size] = x[off:off + size]
    # xt[p, mt, o, j] = xe[mt*128 + j, o*128 + p]
    xt = np.ascontiguousarray(
        xe.reshape(MT, P, KT, P).transpose(3, 0, 2, 1).astype(bf16)
    )
    # wt[p, n, o, j] = w[e][n*512 + j, o*128 + p]
    wt = np.ascontiguousarray(
        w[expert].reshape(NT, NFREE, KT, P).transpose(3, 0, 2, 1).astype(bf16)
    )
    return {"xt": xt, "wt": wt}


def kernel(**inputs) -> np.ndarray:
    x = np.asarray(inputs["input_tokens"], dtype=np.float32)       # [T, K]
    w = np.asarray(inputs["weight_stack"], dtype=np.float32)       # [E, O, K]
    m_sizes = np.asarray(inputs["m_sizes"]).astype(np.int64)       # [E]
    m_offsets = np.asarray(inputs["m_offsets"]).astype(np.int64)   # [E]

    T = x.shape[0]
    E, O, K = w.shape
    assert K == IN_F and O == OUT_F and E == NUM_CORES

    # Split each expert's contiguous token group into chunks of <= CAP rows
    # (the deterministic setup gives exactly one CAP-sized chunk per expert).
    chunks = []  # (expert, src_off, size)
    for e in range(E):
        off, size = int(m_offsets[e]), int(m_sizes[e])
        off = max(0, min(off, T))
        size = max(0, min(size, T - off))
        pos = 0
        while pos < size:
            c = min(CAP, size - pos)
            chunks.append((e, off + pos, c))
            pos += c

    out = np.zeros((T, O), dtype=np.float32)
    run = _get_runner()
    for batch_start in range(0, len(chunks), NUM_CORES):
        batch = chunks[batch_start:batch_start + NUM_CORES]
        in_maps = [_chunk_in_map(x, w, off, size, e) for (e, off, size) in batch]
        # SPMD needs a full complement of cores; pad with repeats of map 0.
        while len(in_maps) < NUM_CORES:
            in_maps.append(in_maps[0])
        results = run(in_maps)
        for i, (e, off, size) in enumerate(batch):
            ye = results[i]["y"]  # [CAP, OUT_F] bf16
            out[off:off + size] += ye[:size].astype(np.float32)
    return out
